# revision 10
# baseline (speedup 1.0000x reference)
"""Trainium2 Bass kernel for nn_BiLinearInteractionLayer.

Math: x:(B=4096, F=32, D=64) f32, W:(P=496, D=64, D=64) f32 (torch Linear
layout: out_e = sum_d in_d * W[e, d]).  For each pair p=(i,j), i<j:
    out[b, p, e] = (sum_d x[b,i,d] * W[p,e,d]) * x[b,j,e]

Strategy (data-parallel over batch, 8 cores x 512 rows):

The kernel is HBM-bound: the f32 output alone is 65 MB/core (~165us at
the ~400 GB/s/core the SDMA fabric sustains).  The correctness gate is
rel_err < 2e-2, so all inputs are shipped as fp16 (measured end-to-end
rel err ~4e-4): single-pass k=64 fp16 matmuls with f32 PSUM accumulate,
no hi/lo split.  x is pre-transposed AND pre-converted on the host, so
the device does ZERO layout work.

Per-core HBM traffic: 2 MB xT (fp16, transposed, tile-contiguous)
+ 2 MB xn (fp16, native, elementwise operand) + 4.06 MB weights (fp16,
pretransposed WT[d, p*64+e]) + 65 MB out = ~73 MB.

Per 128-row batch tile, per left field i (npair = 31-i consecutive pairs):
k=64 matmuls in 8-pair chunks (512 f32 PSUM cols = one bank) accumulate
y = xT_i^T @ WT into a per-field PSUM tile; one elementwise product with
the natively-laid right-field slice xn[:, (i+1)*64:] and one store per
field ship the contiguous pair range.

Fields are processed big/small interleaved - [30, 0, 1, 29, 2, 28, ...,
14, 16, 15] - so each ~1 MB field rides with a small partner: the store
stream stays dense through the whole tile instead of degenerating into
overhead-dominated dribbles at the triangular tail (that tail shape
measured ~20 us/run of DMA idle when fields ran in natural order).

The elementwise product runs on two lanes so no single engine gates the
stores: big fields on DVE tensor_mul straight out of PSUM (1
elem/lane/cyc; fp32-with-PSUM-operand caps DVE at 1x, and ~74% of
elements on DVE keeps it under the DMA roof), and the small partner (4-15
pairs) of each unit on an ACT copy (PSUM->SBUF; ACT is closest to PSUM)
chained with a GPSIMD tensor_mul (GPSIMD has no PSUM port) - ~23% of
elements, one chain op per unit, always small so the 2-stage latency
never backs up PSUM-slot recycling.  The lanes use separate output-tile
tags so slot reuse never couples them.

Queueing: loads ride the scalar (ACT) HWDGE ring, stores the sync (SP)
ring, so stores never head-of-line block loads.  Field 30 (1 pair) is a
pipe-cleaner whose weight column (8 KB) + xT slice (32 KB) + xn tile
load first, so the first store issues after ~0.6 MB instead of after the
full first x-tile + weight group.  Weight groups then grow 1,2,4,5,...
once the store stream is flowing.
"""
import numpy as np

import concourse.bacc as bacc
import concourse.tile as tile
import concourse.mybir as mybir
from concourse.bass_utils import run_bass_kernel_spmd

B = 4096
F = 32
D = 64
P = F * (F - 1) // 2  # 496
N_CORES = 8
BL = B // N_CORES     # 512 rows per core
BT = 128              # batch tile (SBUF partitions)
NBT = BL // BT        # 4 batch tiles per core
CHUNK = 8             # pairs per matmul chunk (8*64 = 512 = one PSUM bank)
NLEFT = F - 1         # left fields 0..30

# weight-load groups over fields 0..29 (field 30 has its own tiny tile):
# tiny first groups so compute starts while the bulk streams in.  NOTE:
# fields are consumed interleaved [0, 1, 29, 2, 28, ...], so late fields
# (29, 28, ...) are needed EARLY - their weights sit in groups 1-3.
_GROUP_SIZES = [1, 2, 4, 5, 5, 5, 4, 4]
_GROUPS = []
_g0 = 0
for _gs in _GROUP_SIZES:
    _GROUPS.append((_g0, _gs))
    _g0 += _gs
assert _g0 == NLEFT - 1  # fields 0..29

# big/small interleaved processing order (see module docstring)
_ORDER = [30, 0]
for _k in range(1, 15):
    _ORDER += [_k, 30 - _k]
_ORDER += [15]
assert sorted(_ORDER) == list(range(31))

# chain-lane fields: the small partner of each unit, 4..15 pairs each
# (fields 16..27), ~23% of elements
_CHAIN = set(range(16, 28))

f32 = mybir.dt.float32
f16 = mybir.dt.float16

_nc_cache = None


def _off(i):
    """Pair index of the first pair with left field i."""
    return 31 * i - i * (i - 1) // 2


def _field_group(i):
    for gi, (g0, gn) in enumerate(_GROUPS):
        if g0 <= i < g0 + gn:
            return gi
    raise ValueError(i)


def _build():
    nc = bacc.Bacc("TRN2", target_bir_lowering=False, debug=False,
                   num_devices=N_CORES)
    # xt[d, bt*(F*BT) + f*BT + b] = fp16(x[bt*BT+b, f, d]); each batch
    # tile's slab is contiguous -> one clean [64, 8KB] load per tile
    xt_in = nc.dram_tensor("xt", [D, NBT * F * BT], f16,
                           kind="ExternalInput").ap()
    xn_in = nc.dram_tensor("xn", [BL, F * D], f16, kind="ExternalInput").ap()
    # wt[d, p*64+e] = fp16(W[p, e, d])
    wt_in = nc.dram_tensor("wt", [D, P * D], f16, kind="ExternalInput").ap()
    out = nc.dram_tensor("out", [BL, P * D], f32, kind="ExternalOutput").ap()

    with tile.TileContext(nc) as tc:
        with (
            tc.tile_pool(name="consts", bufs=1) as consts,
            tc.tile_pool(name="xtp", bufs=2) as xtp,
            tc.tile_pool(name="xnp", bufs=2) as xnp,
            tc.tile_pool(name="otp", bufs=1) as otp,
            tc.tile_pool(name="tmp", bufs=2) as tmpp,
            tc.tile_pool(name="psm", bufs=2, space="PSUM") as psm,
        ):
            wt_f30 = consts.tile([D, D], f16, tag="wt_f30")
            wt_g = []
            for gi, (g0, gn) in enumerate(_GROUPS):
                c0 = _off(g0) * D
                c1 = _off(g0 + gn) * D
                t = consts.tile([D, c1 - c0], f16, tag=f"wt{gi}")
                wt_g.append(t)
            # bt0 fast-path xT columns for fields 30 and 0, and the 64-col
            # xn slice field 30's elementwise product needs
            xt0a = consts.tile([D, 2, BT], f16, tag="xt0a")
            xn0a = consts.tile([BT, D], f16, tag="xn0a")

            for bt in range(NBT):
                rows = slice(bt * BT, (bt + 1) * BT)
                if bt == 0:
                    # critical path to the first store: field 30's weight
                    # column + its xT slice + its 64-col xn slice — ~56 KB
                    # total before the first store can go
                    nc.scalar.dma_start(out=wt_f30,
                                        in_=wt_in[:, _off(30) * D:P * D])
                    nc.scalar.dma_start(out=xt0a[:, 0, :],
                                        in_=xt_in[:, 30 * BT:31 * BT])
                    nc.scalar.dma_start(out=xn0a,
                                        in_=xn_in[0:BT, 31 * D:F * D])
                    nc.scalar.dma_start(out=xt0a[:, 1, :],
                                        in_=xt_in[:, 0:BT])
                    c0, c1 = 0, _off(1) * D
                    nc.scalar.dma_start(out=wt_g[0], in_=wt_in[:, c0:c1])
                xn_tile = xnp.tile([BT, F * D], f16, tag="xn")
                nc.scalar.dma_start(out=xn_tile, in_=xn_in[rows, :])
                xt_tile = xtp.tile([D, F * BT], f16, tag="xt")
                nc.scalar.dma_start(
                    out=xt_tile,
                    in_=xt_in[:, bt * F * BT:(bt + 1) * F * BT])
                if bt == 0:
                    # group load order follows the interleaved consumption
                    # order: g7 (fields 26-29) is needed from position 3
                    for gi in [1, 7, 2, 6, 3, 5, 4]:
                        g0, gn = _GROUPS[gi]
                        c0 = _off(g0) * D
                        c1 = _off(g0 + gn) * D
                        nc.scalar.dma_start(out=wt_g[gi], in_=wt_in[:, c0:c1])

                # chain-lane stores are deferred two positions on the SP
                # ring so a GPSIMD mul still in flight never head-of-line
                # blocks the store of an already-finished DVE field
                pending = []

                def flush(idx):
                    while pending and pending[0][0] <= idx - 2:
                        _, st_rows, st_p0, st_np, st_ot = pending.pop(0)
                        nc.sync.dma_start(
                            out=out[st_rows,
                                    st_p0 * D:(st_p0 + st_np) * D],
                            in_=st_ot)

                for idx, i in enumerate(_ORDER):
                    npair = F - 1 - i  # pairs (i, i+1..31), consecutive
                    p0 = _off(i)
                    if i == 30:
                        wtt = wt_f30
                        gbase = p0 * D
                    else:
                        gi = _field_group(i)
                        wtt = wt_g[gi]
                        gbase = _off(_GROUPS[gi][0]) * D
                    if bt == 0 and i in (30, 0):
                        xts = xt0a[:, (0 if i == 30 else 1), :]
                    else:
                        xts = xt_tile[:, i * BT:(i + 1) * BT]
                    pm = psm.tile([BT, npair * D], f32, tag="mm")
                    for c0 in range(0, npair, CHUNK):
                        n = min(CHUNK, npair - c0) * D
                        cs = (p0 + c0) * D - gbase
                        nc.tensor.matmul(
                            pm[:, c0 * D:c0 * D + n], xts,
                            wtt[:, cs:cs + n], start=True, stop=True)
                    if bt == 0 and i == 30:
                        xnsl = xn0a
                    else:
                        xnsl = xn_tile[:, (i + 1) * D:(i + 1 + npair) * D]
                    if i in _CHAIN:
                        # chain lane: ACT moves PSUM to SBUF, GPSIMD does
                        # the product -> DVE stays free for the big fields
                        ot = otp.tile([BT, npair * D], f32, tag="otc",
                                      bufs=3)
                        tm = tmpp.tile([BT, npair * D], f32, tag="tm")
                        nc.scalar.copy(tm, pm)
                        nc.gpsimd.tensor_mul(ot, tm, xnsl)
                        pending.append((idx, rows, p0, npair, ot))
                    else:
                        # fused PSUM->SBUF move + elementwise product
                        ot = otp.tile([BT, npair * D], f32, tag="ot",
                                      bufs=5)
                        nc.vector.tensor_mul(ot, pm, xnsl)
                        nc.sync.dma_start(
                            out=out[rows, p0 * D:(p0 + npair) * D], in_=ot)
                    flush(idx)
                flush(10**9)
    nc.compile()
    return nc


def _get_nc():
    global _nc_cache
    if _nc_cache is None:
        _nc_cache = _build()
    return _nc_cache


def _prep_inputs(x, W):
    x = np.asarray(x, dtype=np.float32)
    W = np.asarray(W, dtype=np.float32)
    wt = np.ascontiguousarray(
        W.transpose(2, 0, 1).reshape(D, P * D).astype(np.float16))
    xs = x.reshape(N_CORES, NBT, BT, F, D)
    # xt[c, d, bt, f, b]
    xt = np.ascontiguousarray(xs.transpose(0, 4, 1, 3, 2)).astype(np.float16)
    xt = xt.reshape(N_CORES, D, NBT * F * BT)
    xn = x.reshape(N_CORES, BL, F * D).astype(np.float16)
    return xt, xn, wt


def _run(x, W, trace=False, trace_kwargs=None):
    xt, xn, wt = _prep_inputs(x, W)
    in_maps = [{"xt": xt[c], "xn": xn[c], "wt": wt}
               for c in range(N_CORES)]
    res = run_bass_kernel_spmd(_get_nc(), in_maps, list(range(N_CORES)),
                               trace=trace, **(trace_kwargs or {}))
    outs = [res.results[c]["out"].reshape(BL, P, D) for c in range(N_CORES)]
    return np.concatenate(outs, axis=0), res


def kernel(x, W):
    out, _ = _run(x, W)
    return out
